# revision 38
# baseline (speedup 1.0000x reference)
"""Trainium2 Bass kernel for nn_AttentionBlock (pre-LN causal attention + SiLU MLP).

8-core SPMD strategy (data-parallel over batch x sequence-parallel over rows):
  - core c handles sample b = c // NPOS, position g = c % NPOS
  - the L rows of a sample are split into NBLK blocks of BS rows; each core owns
    NBPC blocks, paired to balance causal-attention cost (host-chosen pairing)
  - every core computes LN1 + K^T/V for the keys its own blocks attend to
    (replicated across the sample's cores), q/proj/MLP only for its own rows.
    Per-core differences are handled with tc.If branches on partition_id.
  - rows >= mask_len see an all-masked score row; softmax then degenerates to
    the uniform average of V over all L keys.  That average (vbar) and its
    projection pv = vbar @ w_proj depend only on the inputs, so the host
    precomputes pv; the device folds it in as a rank-1 term of the proj matmul
    (moving operand = (1-sel)), with the attention output gated by sel.

All matmul layouts are "transposed" (feature dim on partitions) so no on-device
transposes are needed anywhere; the host feeds x pre-transposed and re-assembles
the transposed output.
"""
import math
from contextlib import ExitStack, nullcontext
from dataclasses import dataclass

import ml_dtypes
import numpy as np

import concourse.bass as bass
import concourse.mybir as mybir
import concourse.tile as tile
from concourse import bacc
from concourse.bass import ds, ts
from concourse.bass_utils import run_bass_kernel_spmd

F32 = mybir.dt.float32
BF16 = mybir.dt.bfloat16
F8 = mybir.dt.float8e4
AF = mybir.ActivationFunctionType
ALU = mybir.AluOpType
BF16NP = ml_dtypes.bfloat16
F8NP = ml_dtypes.float8_e4m3
DR = mybir.MatmulPerfMode.DoubleRow
S8 = 32.0          # fp8 weight pre-scale (undone on PSUM readout)


@dataclass
class Cfg:
    B: int = 2
    L: int = 2048
    E: int = 768
    H: int = 12
    D: int = 64
    FF: int = 3072
    BS: int = 256          # query block rows
    n_cores: int = 8
    eps: float = 1e-6

    @property
    def NPOS(self):
        return self.n_cores // self.B

    @property
    def NBLK(self):
        return self.L // self.BS

    @property
    def NBPC(self):
        return self.NBLK // self.NPOS   # blocks per core

    @property
    def R(self):
        return self.NBPC * self.BS      # own rows per core

    @property
    def EC(self):
        return self.E // 128

    @property
    def FC(self):
        return self.FF // 128

    @property
    def HC(self):
        return self.H // 2              # head-pair chunks (= EC since E = H*D, D=64)


def plan_blocks(cfg: Cfg, mask_lens):
    """Choose jmax (number of attention-active blocks) and block pairing."""
    mmax = int(max(int(m) for m in mask_lens))
    mmax = max(1, min(cfg.L, mmax))
    jmax = (mmax + cfg.BS - 1) // cfg.BS          # blocks [0, jmax) need causal attn
    def cost(j):
        return (j + 1) if j < jmax else 0
    order = sorted(range(cfg.NBLK), key=lambda j: -cost(j))
    pairs = []
    for g in range(cfg.NPOS):
        blocks = []
        for s in range(cfg.NBPC):
            # snake over sorted order: pair heavy with light
            idx = g if s % 2 == 0 else (cfg.NBLK - 1 - g)
            blocks.append(order[idx])
        pairs.append(tuple(blocks))
    return pairs, jmax


def kc_of(cfg: Cfg, j, jmax):
    """number of 128-wide key chunks block j attends to (0 if mask-free)."""
    if j >= jmax:
        return 0
    return (j + 1) * cfg.BS // 128


# ----------------------------------------------------------------------------
# program builder
# ----------------------------------------------------------------------------

def build_program(cfg: Cfg, pairs, jmax, flags, bake_g=None, stage_limit=99, repeat=1,
                  loop_n=1, ablate=(), use_cc=False):
    """flags: dict with bools: bq, bk, bv, bproj, bfc, bout, ln1aff, ln2aff

    bake_g: if set, emit only that variant's attention without tc.If (for
    timing estimation with TimelineSim)."""
    E, L, H, FF, BS, R = cfg.E, cfg.L, cfg.H, cfg.FF, cfg.BS, cfg.R
    EC, FC, HC, NBPC = cfg.EC, cfg.FC, cfg.HC, cfg.NBPC
    KEYS = jmax * BS
    KC = KEYS // 128
    QRS = 1.0 / (math.sqrt(cfg.D) * S8)   # q readout scale (1/sqrt(D) not in wq)

    nc = bacc.Bacc(num_devices=cfg.n_cores)

    # ---- dram I/O ----
    d_xTf = nc.dram_tensor("xT_full", [128, EC * KEYS], BF16, kind="ExternalInput")
    d_xTo = nc.dram_tensor("xT_own", [128, EC * R], BF16, kind="ExternalInput")
    EP = EC // 2       # contraction pair chunks for DoubleRow
    FP = FC // 2
    d_wq = nc.dram_tensor("wq", [128, EC * E], F8, kind="ExternalInput")
    d_wk = nc.dram_tensor("wk", [128, EC * E], F8, kind="ExternalInput")
    d_wv = nc.dram_tensor("wv", [128, EC * E], F8, kind="ExternalInput")
    d_wp = nc.dram_tensor("wproj", [128, EC * E], F8, kind="ExternalInput")
    d_wfc = nc.dram_tensor("wfc", [FC // 4, 128, 4 * EC * 128], F8,
                           kind="ExternalInput")
    d_wout = nc.dram_tensor("wout", [EC, 128, FC * 128], F8,
                            kind="ExternalInput")
    d_bq = nc.dram_tensor("bq", [128, EC], F32, kind="ExternalInput")
    d_bk = nc.dram_tensor("bk", [128, EC], F32, kind="ExternalInput")
    d_bv = nc.dram_tensor("bv", [1, E], BF16, kind="ExternalInput")
    d_bp = nc.dram_tensor("bproj", [128, EC], F32, kind="ExternalInput")
    d_bfc = nc.dram_tensor("bfc", [128, FC], F32, kind="ExternalInput")
    d_bout = nc.dram_tensor("bout", [128, EC], F32, kind="ExternalInput")
    d_ln = nc.dram_tensor("lnp", [128, 4, EC], F32, kind="ExternalInput")  # g1,b1,g2,b2
    d_pv = nc.dram_tensor("pv", [1, E], BF16, kind="ExternalInput")
    d_selb = nc.dram_tensor("selb", [128, R], BF16, kind="ExternalInput")
    d_sel1m = nc.dram_tensor("sel1m", [1, R], BF16, kind="ExternalInput")
    d_masks = nc.dram_tensor("diagmasks", [2, 128, BS], BF16, kind="ExternalInput")
    d_out = nc.dram_tensor("outT", [128, EC * R], BF16, kind="ExternalOutput")
    d_kvloc = d_kvgath = None
    if use_cc:
        d_kvloc = nc.dram_tensor("kvloc", [max(1, (2 * jmax + cfg.NPOS - 1) // cfg.NPOS),
                                           128, EC * 128 + H * 64], BF16)
        d_kvgath = nc.dram_tensor("kvgath",
                                  [cfg.NPOS * max(1, (2 * jmax + cfg.NPOS - 1) // cfg.NPOS),
                                   128, EC * 128 + H * 64], BF16)

    xTf_r = d_xTf.rearrange("p (c n) -> p c n", c=EC)
    out_r = d_out.rearrange("p (c n) -> p c n", c=EC)

    kc_need = [max(kc_of(cfg, j, jmax) for j in pairs[g]) for g in range(cfg.NPOS)]
    act_slots = [[s for s in range(NBPC) if kc_of(cfg, pairs[g][s], jmax) > 0]
                 for g in range(cfg.NPOS)]

    # ---- collective K/V split: each quad member computes ~KC/NPOS key chunks
    # (preferring its own blocks' columns so q needs no extra LN), then the
    # quad AllGathers kT+V via DRAM. ----
    KCMAX = 2 * jmax
    tgt_share = (KCMAX + cfg.NPOS - 1) // cfg.NPOS
    own_chunks = []
    for g in range(cfg.NPOS):
        ch = []
        for s in act_slots[g]:
            j = pairs[g][s]
            ch += [2 * j, 2 * j + 1]
        own_chunks.append(sorted(c for c in ch if c < KCMAX))
    share = [list(c) for c in own_chunks]
    if use_cc:
        moved = True
        while moved:
            moved = False
            over = [g for g in range(cfg.NPOS) if len(share[g]) > tgt_share]
            under = [g for g in range(cfg.NPOS) if len(share[g]) < tgt_share]
            if over and under:
                c = share[over[0]].pop(0)   # donate lowest chunk
                share[under[0]].append(c)
                moved = True
        share = [sorted(s) for s in share]
    extra_ln = [sorted(set(own_chunks[g]) - set(share[g])) for g in range(cfg.NPOS)]
    chunk_owner = {}
    for g in range(cfg.NPOS):
        for slot, ki in enumerate(share[g]):
            chunk_owner[ki] = (g, slot)
    KV_W = EC * 128 + H * 64      # per-chunk payload: kT part + V part

    with tile.TileContext(nc) as tc, ExitStack() as st:
        # ------- persistent tiles (allocated once; re-written each body) -------
        cpool = st.enter_context(tc.tile_pool(name="consts", bufs=1))

        wp_s = cpool.tile([128, EC, E], F8)
        xo_s = cpool.tile([128, EC, R], BF16)
        bq_s = cpool.tile([128, EC], F32)
        bk_s = cpool.tile([128, EC], F32)
        bv_s = cpool.tile([1, E], BF16)
        bp_s = cpool.tile([128, EC], F32)
        bfc_s = cpool.tile([128, FC], F32)
        bout_s = cpool.tile([128, EC], F32)
        ln_s = cpool.tile([128, 4, EC], F32)
        pv_s = cpool.tile([1, E], BF16)
        selb_s = cpool.tile([128, R], BF16)
        sel1m_s = cpool.tile([1, R], BF16)
        maskAB = cpool.tile([128, 2, BS], BF16)
        oinv_col = cpool.tile([128, 1], BF16)    # 1/E for mean matmuls
        ones_row = cpool.tile([1, 128], BF16)
        nones_row = cpool.tile([1, 128], BF16)   # -1
        ones_rf = cpool.tile([1, 64], BF16)
        eps_11 = cpool.tile([1, 1], F32)
        yT = cpool.tile([128, HC, R], BF16)

      # body emitted under For_i (loop_n>1) or `repeat` times; 6-space indent
      # keeps the body indentation valid in both paths.

        def emit_body(ri):
          with tc.tile_pool(name=f"wstream{ri}", bufs=4) as wstream, \
               tc.tile_pool(name=f"wstream2{ri}", bufs=2) as wstream2:
            nc.scalar.dma_start(selb_s[:], d_selb[:])
            nc.scalar.dma_start(sel1m_s[:], d_sel1m[:])
            nc.scalar.dma_start(pv_s[:], d_pv[:])
            nc.scalar.dma_start(maskAB[:], d_masks.rearrange("t p n -> p t n"))
            if flags["bq"]:
                nc.scalar.dma_start(bq_s[:], d_bq[:])
            if flags["bk"]:
                nc.scalar.dma_start(bk_s[:], d_bk[:])
            if flags["bv"]:
                nc.scalar.dma_start(bv_s[:], d_bv[:])
            if flags["bproj"]:
                nc.scalar.dma_start(bp_s[:], d_bp[:])
            if flags["bfc"]:
                nc.scalar.dma_start(bfc_s[:], d_bfc[:])
            if flags["bout"]:
                nc.scalar.dma_start(bout_s[:], d_bout[:])
            if flags["ln1aff"] or flags["ln2aff"]:
                nc.scalar.dma_start(ln_s[:], d_ln[:])
            nc.vector.memset(oinv_col[:], 1.0 / E)
            nc.vector.memset(ones_row[:], 1.0)
            nc.vector.memset(nones_row[:], -1.0)
            nc.vector.memset(ones_rf[:], 1.0)
            nc.vector.memset(eps_11[:], cfg.eps)

            # ============================================================
            # layernorm over a column chunk, transposed layout, in place
            # ============================================================
            def ln_chunk(pool, pspool, bppool, x_bf, cg0, w, aff_idx, tag,
                         dst_of=None):
                """normalize x_bf[:, :, cg0:cg0+w]; the final op per chunk c
                writes dst_of(c) (e.g. an fp8 view) if given, else in place."""
                affine = flags["ln1aff"] if aff_idx == (0, 1) else flags["ln2aff"]
                ps_su = pspool.tile([1, 512], F32, tag="lnp", name=f"su{tag}")
                ps_sq = pspool.tile([1, 512], F32, tag="lnp", name=f"sq{tag}")
                for c in range(EC):
                    nc.tensor.matmul(ps_su[:, :w], oinv_col[:], x_bf[:, c, cg0:cg0 + w],
                                     start=(c == 0), stop=(c == EC - 1))
                sq = pool.tile([128, EC, 512], BF16, tag="lnsq", name=f"sq{tag}")
                nc.vector.tensor_tensor(sq[:, :, :w], x_bf[:, :, cg0:cg0 + w],
                                        x_bf[:, :, cg0:cg0 + w], ALU.mult)
                for c in range(EC):
                    nc.tensor.matmul(ps_sq[:, :w], oinv_col[:], sq[:, c, :w],
                                     start=(c == 0), stop=(c == EC - 1))
                # mu = ps_su ; m2 = ps_sq ; var = m2 - mu^2
                mus = pool.tile([1, 512], F32, tag="lnmus", name=f"mus{tag}")
                nc.scalar.activation(mus[:, :w], ps_su[:, :w], AF.Copy)
                mu2 = pool.tile([1, 512], F32, tag="lnmu2", name=f"m2{tag}")
                nc.vector.tensor_tensor(mu2[:, :w], mus[:, :w], mus[:, :w], ALU.mult)
                va = pool.tile([1, 512], F32, tag="lnva", name=f"va{tag}")
                nc.vector.tensor_tensor(va[:, :w], ps_sq[:, :w], mu2[:, :w], ALU.subtract)
                sd = pool.tile([1, 512], F32, tag="lnsd", name=f"sd{tag}")
                nc.scalar.activation(sd[:, :w], va[:, :w], AF.Sqrt, bias=eps_11[:])
                arow = pool.tile([1, 512], BF16, tag="lnar", name=f"ar{tag}")
                with nc.allow_low_precision(reason="rstd applied in bf16 anyway"):
                    nc.vector.reciprocal(arow[:, :w], sd[:, :w])
                tmu = pool.tile([1, 512], BF16, tag="lntm", name=f"tm{tag}")
                nc.vector.tensor_tensor(tmu[:, :w], mus[:, :w], arow[:, :w], ALU.mult)
                ab = bppool.tile([128, 2, 512], F32, tag="lnab", name=f"ab{tag}")
                nc.tensor.matmul(ab[:, 0, :w], ones_row[:], arow[:, :w],
                                 start=True, stop=True)
                nc.tensor.matmul(ab[:, 1, :w], nones_row[:], tmu[:, :w],
                                 start=True, stop=True)
                gi, bi = aff_idx
                for c in range(EC):
                    dst = dst_of(c) if dst_of is not None else x_bf[:, c, cg0:cg0 + w]
                    nc.vector.tensor_tensor(x_bf[:, c, cg0:cg0 + w],
                                            x_bf[:, c, cg0:cg0 + w], ab[:, 0, :w], ALU.mult)
                    if affine:
                        nc.vector.tensor_tensor(x_bf[:, c, cg0:cg0 + w],
                                                x_bf[:, c, cg0:cg0 + w], ab[:, 1, :w],
                                                ALU.add)
                        nc.vector.tensor_scalar(dst, x_bf[:, c, cg0:cg0 + w],
                                                ln_s[:, gi, c:c + 1], ln_s[:, bi, c:c + 1],
                                                ALU.mult, ALU.add)
                    else:
                        nc.vector.tensor_tensor(dst, x_bf[:, c, cg0:cg0 + w],
                                                ab[:, 1, :w], ALU.add)

            # ------- sample-wide tensors (die after attention) -------
            with tc.tile_pool(name="l2", bufs=1) as l2:
                zT = l2.tile([128, EC, KEYS], BF16, tag="zT", name="zT")
                zf8 = l2.tile([128, EC, KEYS], F8, tag="zf8", name="zf8")
                qTs = l2.tile([128, HC, R], BF16, tag="qTs", name="qTs")
                kTs = l2.tile([128, HC, KEYS], BF16, tag="kTs", name="kTs")
                Vs = l2.tile([128, KC, H, 65], BF16, tag="Vs", name="Vs")
                wq_s = l2.tile([128, EC, E], F8, tag="wq", name="wq")
                wk_s = l2.tile([128, EC, E], F8, tag="wk", name="wk")
                wv_s = l2.tile([128, EC, E], F8, tag="wv", name="wv")

                gvar = None if bake_g is not None else nc.partition_id() % cfg.NPOS

                def variant(g):
                    return nullcontext() if bake_g is not None else tc.If(gvar == g)

                def runs_of(chunks, cap=4):
                    runs = []
                    for c in chunks:
                        if runs and c == runs[-1][0] + runs[-1][1] and runs[-1][1] < cap:
                            runs[-1][1] += 1
                        else:
                            runs.append([c, 1])
                    return [(c0, n * 128) for c0, n in runs]

                # weights on the sync queue; x -> zT on the scalar queue so
                # both streams run on DMA engines concurrently
                nc.sync.dma_start(wk_s[:], d_wk.rearrange("p (c n) -> p c n", c=EC))
                for g in range(cfg.NPOS):
                    if bake_g is not None and g != bake_g:
                        continue
                    with variant(g):
                        if use_cc:
                            lnch = sorted(set(share[g]) | set(extra_ln[g]))
                            for c0, w in runs_of(lnch):
                                n0 = c0 * 128
                                nc.scalar.dma_start(zT[:, :, n0:n0 + w],
                                                    xTf_r[:, :, n0:n0 + w])
                        else:
                            for n0 in range(0, kc_need[g] * 128, 512):
                                w = min(512, kc_need[g] * 128 - n0)
                                nc.scalar.dma_start(zT[:, :, n0:n0 + w],
                                                    xTf_r[:, :, n0:n0 + w])
                nc.sync.dma_start(wv_s[:], d_wv.rearrange("p (c n) -> p c n", c=EC))
                nc.sync.dma_start(wq_s[:], d_wq.rearrange("p (c n) -> p c n", c=EC))
                nc.sync.dma_start(wp_s[:], d_wp.rearrange("p (c n) -> p c n", c=EC))
                nc.scalar.dma_start(xo_s[:], d_xTo.rearrange("p (c n) -> p c n", c=EC))
                nc.vector.memset(Vs[:, :, :, 64:65], 1.0)

                def emit_keys_qkv(pspool, g, gtag, lnpool, lnps, lnbp):
                    """LN1 + kT + V for the first kc_need[g] key chunks, plus q
                    for the active own blocks (taken from zT)."""
                    kc = kc_need[g]
                    ncols = kc * 128
                    for n0 in range(0, ncols, 512):
                        w = min(512, ncols - n0)
                        ln_chunk(lnpool, lnps, lnbp, zT, n0, w, (0, 1), f"f{gtag}{n0}",
                                 dst_of=lambda c, n0=n0, w=w: zf8[:, c, n0:n0 + w])
                        # kT for this chunk
                        for m in range(EC):
                            ps = pspool.tile([128, 512], F32, tag="gp",
                                            name=f"psk{gtag}{m}{n0}")
                            for c in range(EP):
                                nc.tensor.matmul(ps[:, :w],
                                                 wk_s[:, 2 * c:2 * c + 2, ts(m, 128)],
                                                 zf8[:, 2 * c:2 * c + 2, n0:n0 + w],
                                                 start=(c == 0), stop=(c == EP - 1),
                                                 perf_mode=DR)
                            if flags["bk"]:
                                nc.vector.tensor_scalar(kTs[:, m, n0:n0 + w], ps[:, :w],
                                                        1.0 / S8, bk_s[:, m:m + 1],
                                                        ALU.mult, ALU.add)
                            else:
                                nc.scalar.activation(kTs[:, m, n0:n0 + w], ps[:, :w],
                                                     AF.Copy, scale=1.0 / S8)
                        # V rows for this chunk (natural layout, col 64 = 1.0)
                        for r in range(n0 // 128, (n0 + w) // 128):
                            for v0 in range(0, E, 512):
                                vw = min(512, E - v0)
                                ps = pspool.tile([128, 512], F32, tag="gp",
                                                name=f"psv{gtag}{r}{v0}")
                                for c in range(EP):
                                    nc.tensor.matmul(ps[:, :vw],
                                                     zf8[:, 2 * c:2 * c + 2, ts(r, 128)],
                                                     wv_s[:, 2 * c:2 * c + 2, v0:v0 + vw],
                                                     start=(c == 0),
                                                     stop=(c == EP - 1 and not flags["bv"]),
                                                     perf_mode=DR)
                                if flags["bv"]:
                                    nc.tensor.matmul(ps[:, :vw], ones_row[:],
                                                     bv_s[:, v0:v0 + vw],
                                                     start=False, stop=True)
                                h0 = v0 // 64
                                nh = vw // 64
                                nc.scalar.activation(
                                    Vs[:, r, h0:h0 + nh, 0:64],
                                    ps[:, :vw].rearrange("p (h d) -> p h d", d=64),
                                    AF.Copy, scale=1.0 / S8)
                    # q for active own blocks (their columns are already in zf8)
                    for s in act_slots[g]:
                        j = pairs[g][s]
                        for m in range(EC):
                            ps = pspool.tile([128, 512], F32, tag="gp",
                                            name=f"psq{gtag}{s}{m}")
                            for c in range(EP):
                                nc.tensor.matmul(ps[:, :BS],
                                                 wq_s[:, 2 * c:2 * c + 2, ts(m, 128)],
                                                 zf8[:, 2 * c:2 * c + 2,
                                                     j * BS:(j + 1) * BS],
                                                 start=(c == 0), stop=(c == EP - 1),
                                                 perf_mode=DR)
                            if flags["bq"]:
                                nc.vector.tensor_scalar(qTs[:, m, ds(s * BS, BS)],
                                                        ps[:, :BS], QRS, bq_s[:, m:m + 1],
                                                        ALU.mult, ALU.add)
                            else:
                                nc.scalar.activation(qTs[:, m, ds(s * BS, BS)],
                                                     ps[:, :BS], AF.Copy, scale=QRS)
                    # dead own blocks contribute 0 to y*sel; keep them finite
                    for s in range(NBPC):
                        if s not in act_slots[g]:
                            nc.vector.memset(yT[:, :, ds(s * BS, BS)], 0.0)

                def emit_kT_chunk(pspool, gtag, n0, w):
                    for m in range(EC):
                        ps = pspool.tile([128, 512], F32, tag="gp",
                                        name=f"psk{gtag}{m}{n0}")
                        for c in range(EP):
                            nc.tensor.matmul(ps[:, :w],
                                             wk_s[:, 2 * c:2 * c + 2, ts(m, 128)],
                                             zf8[:, 2 * c:2 * c + 2, n0:n0 + w],
                                             start=(c == 0), stop=(c == EP - 1),
                                             perf_mode=DR)
                        if flags["bk"]:
                            nc.vector.tensor_scalar(kTs[:, m, n0:n0 + w], ps[:, :w],
                                                    1.0 / S8, bk_s[:, m:m + 1],
                                                    ALU.mult, ALU.add)
                        else:
                            nc.scalar.activation(kTs[:, m, n0:n0 + w], ps[:, :w],
                                                 AF.Copy, scale=1.0 / S8)

                def emit_V_rows(pspool, gtag, r0, r1):
                    for r in range(r0, r1):
                        for v0 in range(0, E, 512):
                            vw = min(512, E - v0)
                            ps = pspool.tile([128, 512], F32, tag="gp",
                                            name=f"psv{gtag}{r}{v0}")
                            for c in range(EP):
                                nc.tensor.matmul(ps[:, :vw],
                                                 zf8[:, 2 * c:2 * c + 2, ts(r, 128)],
                                                 wv_s[:, 2 * c:2 * c + 2, v0:v0 + vw],
                                                 start=(c == 0),
                                                 stop=(c == EP - 1 and not flags["bv"]),
                                                 perf_mode=DR)
                            if flags["bv"]:
                                nc.tensor.matmul(ps[:, :vw], ones_row[:],
                                                 bv_s[:, v0:v0 + vw],
                                                 start=False, stop=True)
                            nc.scalar.activation(
                                Vs[:, r, v0 // 64:v0 // 64 + vw // 64, 0:64],
                                ps[:, :vw].rearrange("p (h d) -> p h d", d=64),
                                AF.Copy, scale=1.0 / S8)

                def emit_q_deadblocks(pspool, g, gtag):
                    for s in act_slots[g]:
                        j = pairs[g][s]
                        for m in range(EC):
                            ps = pspool.tile([128, 512], F32, tag="gp",
                                            name=f"psq{gtag}{s}{m}")
                            for c in range(EP):
                                nc.tensor.matmul(ps[:, :BS],
                                                 wq_s[:, 2 * c:2 * c + 2, ts(m, 128)],
                                                 zf8[:, 2 * c:2 * c + 2,
                                                     j * BS:(j + 1) * BS],
                                                 start=(c == 0), stop=(c == EP - 1),
                                                 perf_mode=DR)
                            if flags["bq"]:
                                nc.vector.tensor_scalar(qTs[:, m, ds(s * BS, BS)],
                                                        ps[:, :BS], QRS, bq_s[:, m:m + 1],
                                                        ALU.mult, ALU.add)
                            else:
                                nc.scalar.activation(qTs[:, m, ds(s * BS, BS)],
                                                     ps[:, :BS], AF.Copy, scale=QRS)
                    for s in range(NBPC):
                        if s not in act_slots[g]:
                            nc.vector.memset(yT[:, :, ds(s * BS, BS)], 0.0)

                def emit_keys_qkv_cc(pspool, g, gtag, lnpool, lnps, lnbp):
                    lnch = sorted(set(share[g]) | set(extra_ln[g]))
                    for c0, w in runs_of(lnch):
                        ln_chunk(lnpool, lnps, lnbp, zT, c0 * 128, w, (0, 1),
                                 f"f{gtag}{c0}",
                                 dst_of=lambda c, n0=c0 * 128, w=w: zf8[:, c, n0:n0 + w])
                    for c0, w in runs_of(share[g]):
                        emit_kT_chunk(pspool, gtag, c0 * 128, w)
                        emit_V_rows(pspool, gtag, c0, c0 + w // 128)
                    emit_q_deadblocks(pspool, g, gtag)
                    # ship own chunks to DRAM for the quad AllGather
                    for slot, ki in enumerate(share[g]):
                        dst = d_kvloc[slot]
                        nc.sync.dma_start(
                            dst[:, 0:E].rearrange("p (c n) -> p c n", c=EC),
                            kTs[:, :, ts(ki, 128)])
                        nc.sync.dma_start(
                            dst[:, E:E + H * 64].rearrange("p (h d) -> p h d", d=64),
                            Vs[:, ki, :, 0:64])

                def emit_readback(g, gtag):
                    for ki in range(kc_need[g]):
                        if ki in share[g]:
                            continue
                        o, slot = chunk_owner[ki]
                        src_ap = d_kvgath[o * tgt_share + slot]
                        nc.sync.dma_start(
                            kTs[:, :, ts(ki, 128)],
                            src_ap[:, 0:E].rearrange("p (c n) -> p c n", c=EC))
                        nc.sync.dma_start(
                            Vs[:, ki, :, 0:64],
                            src_ap[:, E:E + H * 64].rearrange("p (h d) -> p h d", d=64))

                def emit_attention(g, gtag, att, spsum, ypsum):
                    for slot in act_slots[g]:
                        j = pairs[g][slot]
                        kc = kc_of(cfg, j, jmax)
                        qsl = ds(slot * BS, BS)
                        for hp in range(HC):
                            ps_ys = []
                            for h01 in (0, 1):
                                ps_y = ypsum.tile([65, BS], F32, tag="y",
                                                  name=f"y{gtag}{slot}{hp}{h01}")
                                ps_ys.append(ps_y)
                            kdone = 0
                            while kdone < kc:
                                gsz = min(4, kc - kdone)
                                exs = []
                                for h01 in (0, 1):
                                    pb = h01 * 64
                                    ps_s = spsum.tile([128, 4, BS], F32, tag="s",
                                                      name=f"s{gtag}{slot}{hp}{h01}{kdone}")
                                    for i in range(gsz):
                                        ki = kdone + i
                                        nc.tensor.matmul(
                                            ps_s[:, i, :],
                                            kTs[pb:pb + 64, hp, ts(ki, 128)],
                                            qTs[pb:pb + 64, hp, qsl],
                                            start=True, stop=True)
                                    ex = att.tile([128, 4, BS], BF16, tag="ex",
                                                  name=f"ex{gtag}{slot}{hp}{h01}{kdone}")
                                    nc.scalar.activation(ex[:, :gsz, :], ps_s[:, :gsz, :],
                                                         AF.Exp)
                                    if kdone + gsz == kc:
                                        p0 = kc - 2 - kdone
                                        nc.vector.tensor_tensor(
                                            ex[:, p0:p0 + 2, :], ex[:, p0:p0 + 2, :],
                                            maskAB[:], ALU.mult)
                                    exs.append(ex)
                                # AV after BOTH halves' scores: exp latency hides
                                # under the other half's score matmuls (PE queue
                                # is in-order; AV first would head-of-line block)
                                for h01 in (0, 1):
                                    h = 2 * hp + h01
                                    for i in range(gsz):
                                        ki = kdone + i
                                        nc.tensor.matmul(
                                            ps_ys[h01][:],
                                            Vs[:, ki, h, :],
                                            exs[h01][:, i, :],
                                            start=(ki == 0), stop=(ki == kc - 1))
                                kdone += gsz
                            for h01 in (0, 1):
                                pb = h01 * 64
                                rr = att.tile([1, BS], BF16, tag="rr",
                                              name=f"rr{gtag}{slot}{hp}{h01}")
                                with nc.allow_low_precision(
                                        reason="softmax denom applied in bf16"):
                                    nc.vector.reciprocal(rr[:], ps_ys[h01][64:65, :])
                                rbp = spsum.tile([128, 4, BS], F32, tag="s",
                                                 name=f"rb{gtag}{slot}{hp}{h01}")
                                nc.tensor.matmul(rbp[0:64, 0, :], ones_rf[:], rr[:],
                                                 start=True, stop=True)
                                rbs = att.tile([64, BS], BF16, tag="rbs",
                                               name=f"rbs{gtag}{slot}{hp}{h01}")
                                nc.scalar.activation(rbs[:], rbp[0:64, 0, :], AF.Copy)
                                nc.vector.tensor_tensor(yT[pb:pb + 64, hp, qsl],
                                                        ps_ys[h01][0:64, :],
                                                        rbs[:], ALU.mult)

                with tc.tile_pool(name=f"l3{ri}", bufs=2) as l3, \
                     tc.tile_pool(name=f"qkvps{ri}", bufs=2, space="PSUM") as qkvps, \
                     tc.tile_pool(name=f"lnps{ri}", bufs=3, space="PSUM") as lnps, \
                     tc.tile_pool(name=f"lnbp{ri}", bufs=1, space="PSUM") as lnbp:
                    for g in range(cfg.NPOS):
                        if bake_g is not None and g != bake_g:
                            continue
                        with variant(g):
                            if use_cc:
                                emit_keys_qkv_cc(qkvps, g, str(g), l3, lnps, lnbp)
                            else:
                                emit_keys_qkv(qkvps, g, str(g), l3, lnps, lnbp)
                if use_cc:
                    nc.gpsimd.collective_compute(
                        "AllGather", mybir.AluOpType.bypass,
                        replica_groups=[[b * cfg.NPOS + i for i in range(cfg.NPOS)]
                                        for b in range(cfg.B)],
                        ins=[d_kvloc[:].opt()], outs=[d_kvgath[:].opt()])
                with tc.tile_pool(name=f"att{ri}", bufs=3) as att, \
                     tc.tile_pool(name=f"sps{ri}", bufs=3, space="PSUM") as spsum, \
                     tc.tile_pool(name=f"yps{ri}", bufs=2, space="PSUM") as ypsum:
                    for g in range(cfg.NPOS):
                        if bake_g is not None and g != bake_g:
                            continue
                        with variant(g):
                            if use_cc:
                                emit_readback(g, str(g))
                            emit_attention(g, str(g), att, spsum, ypsum)

            # ------- proj / LN2 / MLP (uniform across cores) -------
            with tc.tile_pool(name="l2c", bufs=1) as l2c, \
                 tc.tile_pool(name=f"mlpps{ri}", bufs=2, space="PSUM") as gpsum:
                ysel = l2c.tile([128, HC, R], F8)
                x1T = l2c.tile([128, EC, R], F32)
                x1b = l2c.tile([128, EC, R], BF16)
                x1f8 = l2c.tile([128, EC, R], F8)
                hT = l2c.tile([128, FC, R], F8)

                nc.vector.tensor_tensor(
                    ysel[:], yT[:],
                    selb_s[:, None, :].to_broadcast([128, HC, R]), ALU.mult)

                for m in range(EC):
                    ps = gpsum.tile([128, 512], F32, tag="gp", name=f"psp{m}")
                    for c in range(EP):
                        nc.tensor.matmul(ps[:, :R], wp_s[:, 2 * c:2 * c + 2, ts(m, 128)],
                                         ysel[:, 2 * c:2 * c + 2, :],
                                         start=(c == 0), stop=False, perf_mode=DR)
                    # masked rows: += pv (x) (1-sel)   [rank-1; pv pre-scaled xS8]
                    nc.tensor.matmul(ps[:, :R], pv_s[0:1, ts(m, 128)], sel1m_s[:],
                                     start=False, stop=True)
                    nc.vector.scalar_tensor_tensor(x1T[:, m, :], ps[:, :R], 1.0 / S8,
                                                   xo_s[:, m, :], ALU.mult, ALU.add)
                    if flags["bproj"]:
                        nc.vector.tensor_scalar(x1T[:, m, :], x1T[:, m, :],
                                                bp_s[:, m:m + 1], None, ALU.add)
                    nc.gpsimd.tensor_copy(x1b[:, m, :], x1T[:, m, :])

                with tc.tile_pool(name="l3c", bufs=2) as l3c, \
                     tc.tile_pool(name=f"lnps2{ri}", bufs=3, space="PSUM") as lnps2, \
                     tc.tile_pool(name=f"lnbp2{ri}", bufs=1, space="PSUM") as lnbp2:
                    for cg0 in range(0, R, 512):
                        w = min(512, R - cg0)
                        ln_chunk(l3c, lnps2, lnbp2, x1b, cg0, w, (2, 3), f"2{cg0}",
                                 dst_of=lambda c, n0=cg0, w=w: x1f8[:, c, n0:n0 + w])

                for mg in range(FC // 4):
                    wfc_m4 = wstream.tile([128, 4 * EC, 128], F8, tag="wfc",
                                          name=f"wfc{mg}")
                    nc.sync.dma_start(
                        wfc_m4[:], d_wfc[mg].rearrange("p (f n) -> p f n", f=4 * EC))
                    for mi in range(4):
                        m = mg * 4 + mi
                        ps = gpsum.tile([128, 512], F32, tag="gp", name=f"psh{m}")
                        for c in range(EP):
                            nc.tensor.matmul(
                                ps[:, :R],
                                wfc_m4[:, mi * EC + 2 * c:mi * EC + 2 * c + 2, :],
                                x1f8[:, 2 * c:2 * c + 2, :],
                                start=(c == 0), stop=(c == EP - 1), perf_mode=DR)
                        if flags["bfc"]:
                            nc.scalar.activation(hT[:, m, :], ps[:, :R], AF.Silu,
                                                 bias=bfc_s[:, m:m + 1], scale=1.0 / S8)
                        else:
                            nc.scalar.activation(hT[:, m, :], ps[:, :R], AF.Silu,
                                                 scale=1.0 / S8)
                for m in range(EC):
                    wout_m = wstream2.tile([128, FC, 128], F8, tag="wout", name=f"wout{m}")
                    nc.scalar.dma_start(wout_m[:], d_wout[m].rearrange("p (k n) -> p k n", k=FC))
                    ps = gpsum.tile([128, 512], F32, tag="gp", name=f"pso{m}")
                    for k in range(FP):
                        nc.tensor.matmul(ps[:, :R], wout_m[:, 2 * k:2 * k + 2, :],
                                         hT[:, 2 * k:2 * k + 2, :],
                                         start=(k == 0), stop=(k == FP - 1),
                                         perf_mode=DR)
                    ot = wstream2.tile([128, 512], BF16, tag="ot", name=f"ot{m}")
                    nc.vector.scalar_tensor_tensor(ot[:, :R], ps[:, :R], 1.0 / S8,
                                                   x1T[:, m, :], ALU.mult, ALU.add)
                    if flags["bout"]:
                        nc.vector.tensor_scalar(ot[:, :R], ot[:, :R],
                                                bout_s[:, m:m + 1], None, ALU.add)
                    nc.gpsimd.dma_start(out_r[:, m, :], ot[:, :R])

        if loop_n > 1:
            with tc.For_i(0, loop_n, 1):
                emit_body(0)
        else:
            for _ri in range(repeat):
                emit_body(_ri)

    nc.finalize()
    return nc


# ----------------------------------------------------------------------------
# host side: input prep / output assembly
# ----------------------------------------------------------------------------

def prepare_in_maps(cfg: Cfg, pairs, jmax, flags, inputs):
    """Build per-core input maps. Returns (in_maps, percore_blocks)."""
    x = np.asarray(inputs["x"], np.float32)
    w_qkv = np.asarray(inputs["w_qkv"], np.float32)
    b_qkv = np.asarray(inputs["b_qkv"], np.float32)
    w_proj = np.asarray(inputs["w_proj"], np.float32)
    b_proj = np.asarray(inputs["b_proj"], np.float32)
    w_fc = np.asarray(inputs["w_fc"], np.float32)
    b_fc = np.asarray(inputs["b_fc"], np.float32)
    w_out = np.asarray(inputs["w_out"], np.float32)
    b_out = np.asarray(inputs["b_out"], np.float32)
    ln1_s = np.asarray(inputs["ln1_scale"], np.float32)
    ln1_b = np.asarray(inputs["ln1_bias"], np.float32)
    ln2_s = np.asarray(inputs["ln2_scale"], np.float32)
    ln2_b = np.asarray(inputs["ln2_bias"], np.float32)
    mask_len = np.asarray(inputs["mask_len"]).astype(np.int64)

    E, L, H, D, BS = cfg.E, cfg.L, cfg.H, cfg.D, cfg.BS
    EC, FC = cfg.EC, cfg.FC
    KEYS = jmax * BS
    qscale = 1.0 / math.sqrt(D)

    # split qkv columns: col = h*3D + {0..D-1:q, D..2D-1:k, 2D..3D-1:v}
    # wq does NOT fold qscale (fp8 subnormals) -- applied on device readout
    wsplit = w_qkv.reshape(E, H, 3 * D)
    wq = np.ascontiguousarray(wsplit[:, :, 0:D].reshape(E, E))
    wk = np.ascontiguousarray(wsplit[:, :, D:2 * D].reshape(E, E))
    wv = np.ascontiguousarray(wsplit[:, :, 2 * D:3 * D].reshape(E, E))
    bsplit = b_qkv.reshape(H, 3 * D)
    bq = np.ascontiguousarray(bsplit[:, 0:D].reshape(E)) * qscale
    bk = np.ascontiguousarray(bsplit[:, D:2 * D].reshape(E))
    bv = np.ascontiguousarray(bsplit[:, 2 * D:3 * D].reshape(E))

    S8f = np.float32(S8)

    def chunked_w(w):  # [E, N] -> partition-major [128, EC*N] fp8 (pre-scaled)
        n = w.shape[1]
        return np.ascontiguousarray(
            (w * S8f).reshape(EC, 128, n).transpose(1, 0, 2)
            .reshape(128, EC * n)).astype(F8NP)

    def col_f32(v):    # [E or FF] -> [128, C]
        return np.ascontiguousarray(v.reshape(-1, 128).T).astype(np.float32)

    wq_c, wk_c, wv_c, wp_c = (chunked_w(w) for w in (wq, wk, wv, w_proj))
    wfc_c = np.ascontiguousarray(
        (w_fc * S8f).reshape(EC, 128, FC, 128).transpose(2, 1, 0, 3)
        .reshape(FC // 4, 4, 128, EC * 128).transpose(0, 2, 1, 3)
        .reshape(FC // 4, 128, 4 * EC * 128)
    ).astype(F8NP)
    wout_c = np.ascontiguousarray(
        (w_out * S8f).reshape(FC, 128, EC, 128).transpose(2, 1, 0, 3)
        .reshape(EC, 128, FC * 128)
    ).astype(F8NP)
    lnp = np.ascontiguousarray(np.stack(
        [col_f32(ln1_s), col_f32(ln1_b), col_f32(ln2_s), col_f32(ln2_b)]
    ).transpose(1, 0, 2))

    ki = np.arange(128)[:, None]
    qi = np.arange(BS)[None, :]
    masks = np.stack([(qi >= ki), (qi >= ki + 128)]).astype(BF16NP)

    # host-side vbar: attention output for fully-masked rows is the uniform
    # average of V over all L keys; pv = vbar @ w_proj folds into proj.
    mu = x.mean(-1, keepdims=True)
    var = ((x - mu) ** 2).mean(-1, keepdims=True)
    z = (x - mu) / np.sqrt(var + 1e-6) * ln1_s + ln1_b          # (B,L,E)
    vbar = z.mean(axis=1) @ wv + bv                              # (B,E)
    pv = vbar @ w_proj                                           # (B,E)

    shared = dict(
        wq=wq_c, wk=wk_c, wv=wv_c, wproj=wp_c, wfc=wfc_c, wout=wout_c,
        bq=col_f32(bq), bk=col_f32(bk),
        bv=(bv * S8f).reshape(1, E).astype(BF16NP),   # V psum is S8-scaled
        bproj=col_f32(b_proj), bfc=col_f32(b_fc), bout=col_f32(b_out),
        lnp=lnp, diagmasks=masks,
    )

    in_maps = []
    percore_blocks = []
    for c in range(cfg.n_cores):
        b = c // cfg.NPOS
        g = c % cfg.NPOS
        blocks = pairs[g]
        percore_blocks.append((b, blocks))
        xT = x[b].T  # [E, L]
        own_cols = np.concatenate(
            [np.arange(j * BS, (j + 1) * BS) for j in blocks])
        sel = (own_cols < mask_len[b]).astype(BF16NP)
        selb = np.broadcast_to(sel[None, :], (128, cfg.R))
        m = dict(shared)
        xk = xT[:, :KEYS]
        m["xT_full"] = np.ascontiguousarray(
            xk.reshape(EC, 128, KEYS).transpose(1, 0, 2).reshape(128, EC * KEYS)
        ).astype(BF16NP)
        xo = xT[:, own_cols]
        m["xT_own"] = np.ascontiguousarray(
            xo.reshape(EC, 128, -1).transpose(1, 0, 2).reshape(128, -1)).astype(BF16NP)
        m["selb"] = np.ascontiguousarray(selb)
        m["sel1m"] = np.ascontiguousarray(
            (1.0 - sel.astype(np.float32)).reshape(1, cfg.R)).astype(BF16NP)
        m["pv"] = np.ascontiguousarray(pv[b].reshape(1, E) * S8).astype(BF16NP)
        in_maps.append(m)
    return in_maps, percore_blocks


def assemble_output(cfg: Cfg, results, percore_blocks):
    out = np.zeros((cfg.B, cfg.L, cfg.E), np.float32)
    for c, res in enumerate(results):
        b, blocks = percore_blocks[c]
        oT = np.asarray(res["outT"], np.float32).reshape(
            128, cfg.EC, cfg.R).transpose(1, 0, 2).reshape(cfg.E, cfg.R)
        for s, j in enumerate(blocks):
            out[b, j * cfg.BS:(j + 1) * cfg.BS, :] = oT[:, s * cfg.BS:(s + 1) * cfg.BS].T
    return out


def make_flags(inputs):
    def nz(name):
        return bool(np.any(np.asarray(inputs[name]) != 0))
    return dict(
        bq=nz("b_qkv"), bk=nz("b_qkv"), bv=nz("b_qkv"),
        bproj=nz("b_proj"), bfc=nz("b_fc"), bout=nz("b_out"),
        ln1aff=bool(np.any(np.asarray(inputs["ln1_scale"]) != 1)
                    or np.any(np.asarray(inputs["ln1_bias"]) != 0)),
        ln2aff=bool(np.any(np.asarray(inputs["ln2_scale"]) != 1)
                    or np.any(np.asarray(inputs["ln2_bias"]) != 0)),
    )


_cached = {}


def kernel(**inputs) -> np.ndarray:
    cfg = Cfg()
    mask_len = np.asarray(inputs["mask_len"]).astype(np.int64)
    pairs, jmax = plan_blocks(cfg, mask_len)
    flags = make_flags(inputs)
    key = (tuple(map(tuple, pairs)), jmax, tuple(sorted(flags.items())), "v4")
    if key not in _cached:
        _cached[key] = build_program(cfg, pairs, jmax, flags, use_cc=False)
    nc = _cached[key]
    in_maps, percore_blocks = prepare_in_maps(cfg, pairs, jmax, flags, inputs)
    r = run_bass_kernel_spmd(nc, in_maps, core_ids=list(range(cfg.n_cores)))
    return assemble_output(cfg, r.results, percore_blocks)


if __name__ == "__main__":
    pass



# revision 39
# speedup vs baseline: 16260.0125x; 16260.0125x over previous
"""Trainium2 Bass kernel for nn_AttentionBlock (pre-LN causal attention + SiLU MLP).

8-core SPMD strategy (data-parallel over batch x sequence-parallel over rows):
  - core c handles sample b = c // NPOS, position g = c % NPOS
  - the L rows of a sample are split into NBLK blocks of BS rows; each core owns
    NBPC blocks, paired to balance causal-attention cost (host-chosen pairing)
  - every core computes LN1 + K^T/V for the keys its own blocks attend to
    (replicated across the sample's cores), q/proj/MLP only for its own rows.
    Per-core differences are handled with tc.If branches on partition_id.
  - rows >= mask_len see an all-masked score row; softmax then degenerates to
    the uniform average of V over all L keys.  That average (vbar) and its
    projection pv = vbar @ w_proj depend only on the inputs, so the host
    precomputes pv; the device folds it in as a rank-1 term of the proj matmul
    (moving operand = (1-sel)), with the attention output gated by sel.

All matmul layouts are "transposed" (feature dim on partitions) so no on-device
transposes are needed anywhere; the host feeds x pre-transposed and re-assembles
the transposed output.
"""
import math
from contextlib import ExitStack, nullcontext
from dataclasses import dataclass

import ml_dtypes
import numpy as np

import concourse.bass as bass
import concourse.mybir as mybir
import concourse.tile as tile
from concourse import bacc
from concourse.bass import ds, ts
from concourse.bass_utils import run_bass_kernel_spmd

F32 = mybir.dt.float32
BF16 = mybir.dt.bfloat16
F8 = mybir.dt.float8e4
AF = mybir.ActivationFunctionType
ALU = mybir.AluOpType
BF16NP = ml_dtypes.bfloat16
F8NP = ml_dtypes.float8_e4m3
DR = mybir.MatmulPerfMode.DoubleRow
S8 = 32.0          # fp8 weight pre-scale (undone on PSUM readout)


@dataclass
class Cfg:
    B: int = 2
    L: int = 2048
    E: int = 768
    H: int = 12
    D: int = 64
    FF: int = 3072
    BS: int = 256          # query block rows
    n_cores: int = 8
    eps: float = 1e-6

    @property
    def NPOS(self):
        return self.n_cores // self.B

    @property
    def NBLK(self):
        return self.L // self.BS

    @property
    def NBPC(self):
        return self.NBLK // self.NPOS   # blocks per core

    @property
    def R(self):
        return self.NBPC * self.BS      # own rows per core

    @property
    def EC(self):
        return self.E // 128

    @property
    def FC(self):
        return self.FF // 128

    @property
    def HC(self):
        return self.H // 2              # head-pair chunks (= EC since E = H*D, D=64)


def plan_blocks(cfg: Cfg, mask_lens):
    """Choose jmax (number of attention-active blocks) and block pairing."""
    mmax = int(max(int(m) for m in mask_lens))
    mmax = max(1, min(cfg.L, mmax))
    jmax = (mmax + cfg.BS - 1) // cfg.BS          # blocks [0, jmax) need causal attn
    def cost(j):
        return (j + 1) if j < jmax else 0
    order = sorted(range(cfg.NBLK), key=lambda j: -cost(j))
    pairs = []
    for g in range(cfg.NPOS):
        blocks = []
        for s in range(cfg.NBPC):
            # snake over sorted order: pair heavy with light
            idx = g if s % 2 == 0 else (cfg.NBLK - 1 - g)
            blocks.append(order[idx])
        pairs.append(tuple(blocks))
    return pairs, jmax


def kc_of(cfg: Cfg, j, jmax):
    """number of 128-wide key chunks block j attends to (0 if mask-free)."""
    if j >= jmax:
        return 0
    return (j + 1) * cfg.BS // 128


# ----------------------------------------------------------------------------
# program builder
# ----------------------------------------------------------------------------

def build_program(cfg: Cfg, pairs, jmax, flags, bake_g=None, stage_limit=99, repeat=1,
                  loop_n=1, ablate=(), use_cc=False):
    """flags: dict with bools: bq, bk, bv, bproj, bfc, bout, ln1aff, ln2aff

    bake_g: if set, emit only that variant's attention without tc.If (for
    timing estimation with TimelineSim)."""
    E, L, H, FF, BS, R = cfg.E, cfg.L, cfg.H, cfg.FF, cfg.BS, cfg.R
    EC, FC, HC, NBPC = cfg.EC, cfg.FC, cfg.HC, cfg.NBPC
    KEYS = jmax * BS
    KC = KEYS // 128
    QRS = 1.0 / (math.sqrt(cfg.D) * S8)   # q readout scale (1/sqrt(D) not in wq)

    nc = bacc.Bacc(num_devices=cfg.n_cores)

    # ---- dram I/O ----
    d_xTf = nc.dram_tensor("xT_full", [128, EC * KEYS], BF16, kind="ExternalInput")
    d_xTo = nc.dram_tensor("xT_own", [128, EC * R], BF16, kind="ExternalInput")
    EP = EC // 2       # contraction pair chunks for DoubleRow
    FP = FC // 2
    d_wq = nc.dram_tensor("wq", [128, EC * E], F8, kind="ExternalInput")
    d_wk = nc.dram_tensor("wk", [128, EC * E], F8, kind="ExternalInput")
    d_wv = nc.dram_tensor("wv", [128, EC * E], F8, kind="ExternalInput")
    d_wp = nc.dram_tensor("wproj", [128, EC * E], F8, kind="ExternalInput")
    d_wfc = nc.dram_tensor("wfc", [FC // 4, 128, 4 * EC * 128], F8,
                           kind="ExternalInput")
    d_wout = nc.dram_tensor("wout", [EC, 128, FC * 128], BF16,
                            kind="ExternalInput")
    d_bq = nc.dram_tensor("bq", [128, EC], F32, kind="ExternalInput")
    d_bk = nc.dram_tensor("bk", [128, EC], F32, kind="ExternalInput")
    d_bv = nc.dram_tensor("bv", [1, E], BF16, kind="ExternalInput")
    d_bp = nc.dram_tensor("bproj", [128, EC], F32, kind="ExternalInput")
    d_bfc = nc.dram_tensor("bfc", [128, FC], F32, kind="ExternalInput")
    d_bout = nc.dram_tensor("bout", [128, EC], F32, kind="ExternalInput")
    d_ln = nc.dram_tensor("lnp", [128, 4, EC], F32, kind="ExternalInput")  # g1,b1,g2,b2
    d_pv = nc.dram_tensor("pv", [1, E], BF16, kind="ExternalInput")
    d_selb = nc.dram_tensor("selb", [128, R], BF16, kind="ExternalInput")
    d_sel1m = nc.dram_tensor("sel1m", [1, R], BF16, kind="ExternalInput")
    d_masks = nc.dram_tensor("diagmasks", [2, 128, BS], BF16, kind="ExternalInput")
    d_out = nc.dram_tensor("outT", [128, EC * R], BF16, kind="ExternalOutput")
    d_kvloc = d_kvgath = None
    if use_cc:
        d_kvloc = nc.dram_tensor("kvloc", [max(1, (2 * jmax + cfg.NPOS - 1) // cfg.NPOS),
                                           128, EC * 128 + H * 64], BF16)
        d_kvgath = nc.dram_tensor("kvgath",
                                  [cfg.NPOS * max(1, (2 * jmax + cfg.NPOS - 1) // cfg.NPOS),
                                   128, EC * 128 + H * 64], BF16)

    xTf_r = d_xTf.rearrange("p (c n) -> p c n", c=EC)
    out_r = d_out.rearrange("p (c n) -> p c n", c=EC)

    kc_need = [max(kc_of(cfg, j, jmax) for j in pairs[g]) for g in range(cfg.NPOS)]
    act_slots = [[s for s in range(NBPC) if kc_of(cfg, pairs[g][s], jmax) > 0]
                 for g in range(cfg.NPOS)]

    # ---- collective K/V split: each quad member computes ~KC/NPOS key chunks
    # (preferring its own blocks' columns so q needs no extra LN), then the
    # quad AllGathers kT+V via DRAM. ----
    KCMAX = 2 * jmax
    tgt_share = (KCMAX + cfg.NPOS - 1) // cfg.NPOS
    own_chunks = []
    for g in range(cfg.NPOS):
        ch = []
        for s in act_slots[g]:
            j = pairs[g][s]
            ch += [2 * j, 2 * j + 1]
        own_chunks.append(sorted(c for c in ch if c < KCMAX))
    share = [list(c) for c in own_chunks]
    if use_cc:
        moved = True
        while moved:
            moved = False
            over = [g for g in range(cfg.NPOS) if len(share[g]) > tgt_share]
            under = [g for g in range(cfg.NPOS) if len(share[g]) < tgt_share]
            if over and under:
                c = share[over[0]].pop(0)   # donate lowest chunk
                share[under[0]].append(c)
                moved = True
        share = [sorted(s) for s in share]
    extra_ln = [sorted(set(own_chunks[g]) - set(share[g])) for g in range(cfg.NPOS)]
    chunk_owner = {}
    for g in range(cfg.NPOS):
        for slot, ki in enumerate(share[g]):
            chunk_owner[ki] = (g, slot)
    KV_W = EC * 128 + H * 64      # per-chunk payload: kT part + V part

    with tile.TileContext(nc) as tc, ExitStack() as st:
        # ------- persistent tiles (allocated once; re-written each body) -------
        cpool = st.enter_context(tc.tile_pool(name="consts", bufs=1))

        wp_s = cpool.tile([128, EC, E], F8)
        xo_s = cpool.tile([128, EC, R], BF16)
        bq_s = cpool.tile([128, EC], F32)
        bk_s = cpool.tile([128, EC], F32)
        bv_s = cpool.tile([1, E], BF16)
        bp_s = cpool.tile([128, EC], F32)
        bfc_s = cpool.tile([128, FC], F32)
        bout_s = cpool.tile([128, EC], F32)
        ln_s = cpool.tile([128, 4, EC], F32)
        pv_s = cpool.tile([1, E], BF16)
        selb_s = cpool.tile([128, R], BF16)
        sel1m_s = cpool.tile([1, R], BF16)
        maskAB = cpool.tile([128, 2, BS], BF16)
        oinv_col = cpool.tile([128, 1], BF16)    # 1/E for mean matmuls
        ones_row = cpool.tile([1, 128], BF16)
        nones_row = cpool.tile([1, 128], BF16)   # -1
        ones_rf = cpool.tile([1, 64], BF16)
        eps_11 = cpool.tile([1, 1], F32)
        yT = cpool.tile([128, HC, R], BF16)

      # body emitted under For_i (loop_n>1) or `repeat` times; 6-space indent
      # keeps the body indentation valid in both paths.

        def emit_body(ri):
          with tc.tile_pool(name=f"wstream{ri}", bufs=4) as wstream, \
               tc.tile_pool(name=f"wstream2{ri}", bufs=2) as wstream2:
            nc.scalar.dma_start(selb_s[:], d_selb[:])
            nc.scalar.dma_start(sel1m_s[:], d_sel1m[:])
            nc.scalar.dma_start(pv_s[:], d_pv[:])
            nc.scalar.dma_start(maskAB[:], d_masks.rearrange("t p n -> p t n"))
            if flags["bq"]:
                nc.scalar.dma_start(bq_s[:], d_bq[:])
            if flags["bk"]:
                nc.scalar.dma_start(bk_s[:], d_bk[:])
            if flags["bv"]:
                nc.scalar.dma_start(bv_s[:], d_bv[:])
            if flags["bproj"]:
                nc.scalar.dma_start(bp_s[:], d_bp[:])
            if flags["bfc"]:
                nc.scalar.dma_start(bfc_s[:], d_bfc[:])
            if flags["bout"]:
                nc.scalar.dma_start(bout_s[:], d_bout[:])
            if flags["ln1aff"] or flags["ln2aff"]:
                nc.scalar.dma_start(ln_s[:], d_ln[:])
            nc.vector.memset(oinv_col[:], 1.0 / E)
            nc.vector.memset(ones_row[:], 1.0)
            nc.vector.memset(nones_row[:], -1.0)
            nc.vector.memset(ones_rf[:], 1.0)
            nc.vector.memset(eps_11[:], cfg.eps)

            # ============================================================
            # layernorm over a column chunk, transposed layout, in place
            # ============================================================
            def ln_chunk(pool, pspool, bppool, x_bf, cg0, w, aff_idx, tag,
                         dst_of=None):
                """normalize x_bf[:, :, cg0:cg0+w]; the final op per chunk c
                writes dst_of(c) (e.g. an fp8 view) if given, else in place."""
                affine = flags["ln1aff"] if aff_idx == (0, 1) else flags["ln2aff"]
                ps_su = pspool.tile([1, 512], F32, tag="lnp", name=f"su{tag}")
                ps_sq = pspool.tile([1, 512], F32, tag="lnp", name=f"sq{tag}")
                for c in range(EC):
                    nc.tensor.matmul(ps_su[:, :w], oinv_col[:], x_bf[:, c, cg0:cg0 + w],
                                     start=(c == 0), stop=(c == EC - 1))
                sq = pool.tile([128, EC, 512], BF16, tag="lnsq", name=f"sq{tag}")
                nc.vector.tensor_tensor(sq[:, :, :w], x_bf[:, :, cg0:cg0 + w],
                                        x_bf[:, :, cg0:cg0 + w], ALU.mult)
                for c in range(EC):
                    nc.tensor.matmul(ps_sq[:, :w], oinv_col[:], sq[:, c, :w],
                                     start=(c == 0), stop=(c == EC - 1))
                # mu = ps_su ; m2 = ps_sq ; var = m2 - mu^2
                mus = pool.tile([1, 512], F32, tag="lnmus", name=f"mus{tag}")
                nc.scalar.activation(mus[:, :w], ps_su[:, :w], AF.Copy)
                mu2 = pool.tile([1, 512], F32, tag="lnmu2", name=f"m2{tag}")
                nc.vector.tensor_tensor(mu2[:, :w], mus[:, :w], mus[:, :w], ALU.mult)
                va = pool.tile([1, 512], F32, tag="lnva", name=f"va{tag}")
                nc.vector.tensor_tensor(va[:, :w], ps_sq[:, :w], mu2[:, :w], ALU.subtract)
                sd = pool.tile([1, 512], F32, tag="lnsd", name=f"sd{tag}")
                nc.scalar.activation(sd[:, :w], va[:, :w], AF.Sqrt, bias=eps_11[:])
                arow = pool.tile([1, 512], BF16, tag="lnar", name=f"ar{tag}")
                with nc.allow_low_precision(reason="rstd applied in bf16 anyway"):
                    nc.vector.reciprocal(arow[:, :w], sd[:, :w])
                tmu = pool.tile([1, 512], BF16, tag="lntm", name=f"tm{tag}")
                nc.vector.tensor_tensor(tmu[:, :w], mus[:, :w], arow[:, :w], ALU.mult)
                ab = bppool.tile([128, 2, 512], F32, tag="lnab", name=f"ab{tag}")
                nc.tensor.matmul(ab[:, 0, :w], ones_row[:], arow[:, :w],
                                 start=True, stop=True)
                nc.tensor.matmul(ab[:, 1, :w], nones_row[:], tmu[:, :w],
                                 start=True, stop=True)
                gi, bi = aff_idx
                for c in range(EC):
                    dst = dst_of(c) if dst_of is not None else x_bf[:, c, cg0:cg0 + w]
                    nc.vector.tensor_tensor(x_bf[:, c, cg0:cg0 + w],
                                            x_bf[:, c, cg0:cg0 + w], ab[:, 0, :w], ALU.mult)
                    if affine:
                        nc.vector.tensor_tensor(x_bf[:, c, cg0:cg0 + w],
                                                x_bf[:, c, cg0:cg0 + w], ab[:, 1, :w],
                                                ALU.add)
                        nc.vector.tensor_scalar(dst, x_bf[:, c, cg0:cg0 + w],
                                                ln_s[:, gi, c:c + 1], ln_s[:, bi, c:c + 1],
                                                ALU.mult, ALU.add)
                    else:
                        nc.vector.tensor_tensor(dst, x_bf[:, c, cg0:cg0 + w],
                                                ab[:, 1, :w], ALU.add)

            # ------- sample-wide tensors (die after attention) -------
            with tc.tile_pool(name="l2", bufs=1) as l2:
                zT = l2.tile([128, EC, KEYS], BF16, tag="zT", name="zT")
                zf8 = l2.tile([128, EC, KEYS], F8, tag="zf8", name="zf8")
                qTs = l2.tile([128, HC, R], BF16, tag="qTs", name="qTs")
                kTs = l2.tile([128, HC, KEYS], BF16, tag="kTs", name="kTs")
                Vs = l2.tile([128, KC, H, 65], BF16, tag="Vs", name="Vs")
                wq_s = l2.tile([128, EC, E], F8, tag="wq", name="wq")
                wk_s = l2.tile([128, EC, E], F8, tag="wk", name="wk")
                wv_s = l2.tile([128, EC, E], F8, tag="wv", name="wv")

                gvar = None if bake_g is not None else nc.partition_id() % cfg.NPOS

                def variant(g):
                    return nullcontext() if bake_g is not None else tc.If(gvar == g)

                def runs_of(chunks, cap=4):
                    runs = []
                    for c in chunks:
                        if runs and c == runs[-1][0] + runs[-1][1] and runs[-1][1] < cap:
                            runs[-1][1] += 1
                        else:
                            runs.append([c, 1])
                    return [(c0, n * 128) for c0, n in runs]

                # weights on the sync queue; x -> zT on the scalar queue so
                # both streams run on DMA engines concurrently
                nc.sync.dma_start(wk_s[:], d_wk.rearrange("p (c n) -> p c n", c=EC))
                for g in range(cfg.NPOS):
                    if bake_g is not None and g != bake_g:
                        continue
                    with variant(g):
                        if use_cc:
                            lnch = sorted(set(share[g]) | set(extra_ln[g]))
                            for c0, w in runs_of(lnch):
                                n0 = c0 * 128
                                nc.scalar.dma_start(zT[:, :, n0:n0 + w],
                                                    xTf_r[:, :, n0:n0 + w])
                        else:
                            for n0 in range(0, kc_need[g] * 128, 512):
                                w = min(512, kc_need[g] * 128 - n0)
                                nc.scalar.dma_start(zT[:, :, n0:n0 + w],
                                                    xTf_r[:, :, n0:n0 + w])
                nc.sync.dma_start(wv_s[:], d_wv.rearrange("p (c n) -> p c n", c=EC))
                nc.sync.dma_start(wq_s[:], d_wq.rearrange("p (c n) -> p c n", c=EC))
                nc.sync.dma_start(wp_s[:], d_wp.rearrange("p (c n) -> p c n", c=EC))
                nc.scalar.dma_start(xo_s[:], d_xTo.rearrange("p (c n) -> p c n", c=EC))
                nc.vector.memset(Vs[:, :, :, 64:65], 1.0)

                def emit_keys_qkv(pspool, g, gtag, lnpool, lnps, lnbp):
                    """LN1 + kT + V for the first kc_need[g] key chunks, plus q
                    for the active own blocks (taken from zT)."""
                    kc = kc_need[g]
                    ncols = kc * 128
                    for n0 in range(0, ncols, 512):
                        w = min(512, ncols - n0)
                        ln_chunk(lnpool, lnps, lnbp, zT, n0, w, (0, 1), f"f{gtag}{n0}",
                                 dst_of=lambda c, n0=n0, w=w: zf8[:, c, n0:n0 + w])
                        # kT for this chunk
                        for m in range(EC):
                            ps = pspool.tile([128, 512], F32, tag="gp",
                                            name=f"psk{gtag}{m}{n0}")
                            for c in range(EP):
                                nc.tensor.matmul(ps[:, :w],
                                                 wk_s[:, 2 * c:2 * c + 2, ts(m, 128)],
                                                 zf8[:, 2 * c:2 * c + 2, n0:n0 + w],
                                                 start=(c == 0), stop=(c == EP - 1),
                                                 perf_mode=DR)
                            if flags["bk"]:
                                nc.vector.tensor_scalar(kTs[:, m, n0:n0 + w], ps[:, :w],
                                                        1.0 / S8, bk_s[:, m:m + 1],
                                                        ALU.mult, ALU.add)
                            else:
                                nc.scalar.activation(kTs[:, m, n0:n0 + w], ps[:, :w],
                                                     AF.Copy, scale=1.0 / S8)
                        # V rows for this chunk (natural layout, col 64 = 1.0)
                        for r in range(n0 // 128, (n0 + w) // 128):
                            for v0 in range(0, E, 512):
                                vw = min(512, E - v0)
                                ps = pspool.tile([128, 512], F32, tag="gp",
                                                name=f"psv{gtag}{r}{v0}")
                                for c in range(EP):
                                    nc.tensor.matmul(ps[:, :vw],
                                                     zf8[:, 2 * c:2 * c + 2, ts(r, 128)],
                                                     wv_s[:, 2 * c:2 * c + 2, v0:v0 + vw],
                                                     start=(c == 0),
                                                     stop=(c == EP - 1 and not flags["bv"]),
                                                     perf_mode=DR)
                                if flags["bv"]:
                                    nc.tensor.matmul(ps[:, :vw], ones_row[:],
                                                     bv_s[:, v0:v0 + vw],
                                                     start=False, stop=True)
                                h0 = v0 // 64
                                nh = vw // 64
                                nc.scalar.activation(
                                    Vs[:, r, h0:h0 + nh, 0:64],
                                    ps[:, :vw].rearrange("p (h d) -> p h d", d=64),
                                    AF.Copy, scale=1.0 / S8)
                    # q for active own blocks (their columns are already in zf8)
                    for s in act_slots[g]:
                        j = pairs[g][s]
                        for m in range(EC):
                            ps = pspool.tile([128, 512], F32, tag="gp",
                                            name=f"psq{gtag}{s}{m}")
                            for c in range(EP):
                                nc.tensor.matmul(ps[:, :BS],
                                                 wq_s[:, 2 * c:2 * c + 2, ts(m, 128)],
                                                 zf8[:, 2 * c:2 * c + 2,
                                                     j * BS:(j + 1) * BS],
                                                 start=(c == 0), stop=(c == EP - 1),
                                                 perf_mode=DR)
                            if flags["bq"]:
                                nc.vector.tensor_scalar(qTs[:, m, ds(s * BS, BS)],
                                                        ps[:, :BS], QRS, bq_s[:, m:m + 1],
                                                        ALU.mult, ALU.add)
                            else:
                                nc.scalar.activation(qTs[:, m, ds(s * BS, BS)],
                                                     ps[:, :BS], AF.Copy, scale=QRS)
                    # dead own blocks contribute 0 to y*sel; keep them finite
                    for s in range(NBPC):
                        if s not in act_slots[g]:
                            nc.vector.memset(yT[:, :, ds(s * BS, BS)], 0.0)

                def emit_kT_chunk(pspool, gtag, n0, w):
                    for m in range(EC):
                        ps = pspool.tile([128, 512], F32, tag="gp",
                                        name=f"psk{gtag}{m}{n0}")
                        for c in range(EP):
                            nc.tensor.matmul(ps[:, :w],
                                             wk_s[:, 2 * c:2 * c + 2, ts(m, 128)],
                                             zf8[:, 2 * c:2 * c + 2, n0:n0 + w],
                                             start=(c == 0), stop=(c == EP - 1),
                                             perf_mode=DR)
                        if flags["bk"]:
                            nc.vector.tensor_scalar(kTs[:, m, n0:n0 + w], ps[:, :w],
                                                    1.0 / S8, bk_s[:, m:m + 1],
                                                    ALU.mult, ALU.add)
                        else:
                            nc.scalar.activation(kTs[:, m, n0:n0 + w], ps[:, :w],
                                                 AF.Copy, scale=1.0 / S8)

                def emit_V_rows(pspool, gtag, r0, r1):
                    for r in range(r0, r1):
                        for v0 in range(0, E, 512):
                            vw = min(512, E - v0)
                            ps = pspool.tile([128, 512], F32, tag="gp",
                                            name=f"psv{gtag}{r}{v0}")
                            for c in range(EP):
                                nc.tensor.matmul(ps[:, :vw],
                                                 zf8[:, 2 * c:2 * c + 2, ts(r, 128)],
                                                 wv_s[:, 2 * c:2 * c + 2, v0:v0 + vw],
                                                 start=(c == 0),
                                                 stop=(c == EP - 1 and not flags["bv"]),
                                                 perf_mode=DR)
                            if flags["bv"]:
                                nc.tensor.matmul(ps[:, :vw], ones_row[:],
                                                 bv_s[:, v0:v0 + vw],
                                                 start=False, stop=True)
                            nc.scalar.activation(
                                Vs[:, r, v0 // 64:v0 // 64 + vw // 64, 0:64],
                                ps[:, :vw].rearrange("p (h d) -> p h d", d=64),
                                AF.Copy, scale=1.0 / S8)

                def emit_q_deadblocks(pspool, g, gtag):
                    for s in act_slots[g]:
                        j = pairs[g][s]
                        for m in range(EC):
                            ps = pspool.tile([128, 512], F32, tag="gp",
                                            name=f"psq{gtag}{s}{m}")
                            for c in range(EP):
                                nc.tensor.matmul(ps[:, :BS],
                                                 wq_s[:, 2 * c:2 * c + 2, ts(m, 128)],
                                                 zf8[:, 2 * c:2 * c + 2,
                                                     j * BS:(j + 1) * BS],
                                                 start=(c == 0), stop=(c == EP - 1),
                                                 perf_mode=DR)
                            if flags["bq"]:
                                nc.vector.tensor_scalar(qTs[:, m, ds(s * BS, BS)],
                                                        ps[:, :BS], QRS, bq_s[:, m:m + 1],
                                                        ALU.mult, ALU.add)
                            else:
                                nc.scalar.activation(qTs[:, m, ds(s * BS, BS)],
                                                     ps[:, :BS], AF.Copy, scale=QRS)
                    for s in range(NBPC):
                        if s not in act_slots[g]:
                            nc.vector.memset(yT[:, :, ds(s * BS, BS)], 0.0)

                def emit_keys_qkv_cc(pspool, g, gtag, lnpool, lnps, lnbp):
                    lnch = sorted(set(share[g]) | set(extra_ln[g]))
                    for c0, w in runs_of(lnch):
                        ln_chunk(lnpool, lnps, lnbp, zT, c0 * 128, w, (0, 1),
                                 f"f{gtag}{c0}",
                                 dst_of=lambda c, n0=c0 * 128, w=w: zf8[:, c, n0:n0 + w])
                    for c0, w in runs_of(share[g]):
                        emit_kT_chunk(pspool, gtag, c0 * 128, w)
                        emit_V_rows(pspool, gtag, c0, c0 + w // 128)
                    emit_q_deadblocks(pspool, g, gtag)
                    # ship own chunks to DRAM for the quad AllGather
                    for slot, ki in enumerate(share[g]):
                        dst = d_kvloc[slot]
                        nc.sync.dma_start(
                            dst[:, 0:E].rearrange("p (c n) -> p c n", c=EC),
                            kTs[:, :, ts(ki, 128)])
                        nc.sync.dma_start(
                            dst[:, E:E + H * 64].rearrange("p (h d) -> p h d", d=64),
                            Vs[:, ki, :, 0:64])

                def emit_readback(g, gtag):
                    for ki in range(kc_need[g]):
                        if ki in share[g]:
                            continue
                        o, slot = chunk_owner[ki]
                        src_ap = d_kvgath[o * tgt_share + slot]
                        nc.sync.dma_start(
                            kTs[:, :, ts(ki, 128)],
                            src_ap[:, 0:E].rearrange("p (c n) -> p c n", c=EC))
                        nc.sync.dma_start(
                            Vs[:, ki, :, 0:64],
                            src_ap[:, E:E + H * 64].rearrange("p (h d) -> p h d", d=64))

                def emit_attention(g, gtag, att, spsum, ypsum):
                    for slot in act_slots[g]:
                        j = pairs[g][slot]
                        kc = kc_of(cfg, j, jmax)
                        qsl = ds(slot * BS, BS)
                        for hp in range(HC):
                            ps_ys = []
                            for h01 in (0, 1):
                                ps_y = ypsum.tile([65, BS], F32, tag="y",
                                                  name=f"y{gtag}{slot}{hp}{h01}")
                                ps_ys.append(ps_y)
                            kdone = 0
                            while kdone < kc:
                                gsz = min(4, kc - kdone)
                                exs = []
                                for h01 in (0, 1):
                                    pb = h01 * 64
                                    ps_s = spsum.tile([128, 4, BS], F32, tag="s",
                                                      name=f"s{gtag}{slot}{hp}{h01}{kdone}")
                                    for i in range(gsz):
                                        ki = kdone + i
                                        nc.tensor.matmul(
                                            ps_s[:, i, :],
                                            kTs[pb:pb + 64, hp, ts(ki, 128)],
                                            qTs[pb:pb + 64, hp, qsl],
                                            start=True, stop=True)
                                    ex = att.tile([128, 4, BS], BF16, tag="ex",
                                                  name=f"ex{gtag}{slot}{hp}{h01}{kdone}")
                                    nc.scalar.activation(ex[:, :gsz, :], ps_s[:, :gsz, :],
                                                         AF.Exp)
                                    if kdone + gsz == kc:
                                        p0 = kc - 2 - kdone
                                        nc.vector.tensor_tensor(
                                            ex[:, p0:p0 + 2, :], ex[:, p0:p0 + 2, :],
                                            maskAB[:], ALU.mult)
                                    exs.append(ex)
                                # AV after BOTH halves' scores: exp latency hides
                                # under the other half's score matmuls (PE queue
                                # is in-order; AV first would head-of-line block)
                                for h01 in (0, 1):
                                    h = 2 * hp + h01
                                    for i in range(gsz):
                                        ki = kdone + i
                                        nc.tensor.matmul(
                                            ps_ys[h01][:],
                                            Vs[:, ki, h, :],
                                            exs[h01][:, i, :],
                                            start=(ki == 0), stop=(ki == kc - 1))
                                kdone += gsz
                            for h01 in (0, 1):
                                pb = h01 * 64
                                rr = att.tile([1, BS], BF16, tag="rr",
                                              name=f"rr{gtag}{slot}{hp}{h01}")
                                with nc.allow_low_precision(
                                        reason="softmax denom applied in bf16"):
                                    nc.vector.reciprocal(rr[:], ps_ys[h01][64:65, :])
                                rbp = spsum.tile([128, 4, BS], F32, tag="s",
                                                 name=f"rb{gtag}{slot}{hp}{h01}")
                                nc.tensor.matmul(rbp[0:64, 0, :], ones_rf[:], rr[:],
                                                 start=True, stop=True)
                                rbs = att.tile([64, BS], BF16, tag="rbs",
                                               name=f"rbs{gtag}{slot}{hp}{h01}")
                                nc.scalar.activation(rbs[:], rbp[0:64, 0, :], AF.Copy)
                                nc.vector.tensor_tensor(yT[pb:pb + 64, hp, qsl],
                                                        ps_ys[h01][0:64, :],
                                                        rbs[:], ALU.mult)

                with tc.tile_pool(name=f"l3{ri}", bufs=2) as l3, \
                     tc.tile_pool(name=f"qkvps{ri}", bufs=2, space="PSUM") as qkvps, \
                     tc.tile_pool(name=f"lnps{ri}", bufs=3, space="PSUM") as lnps, \
                     tc.tile_pool(name=f"lnbp{ri}", bufs=1, space="PSUM") as lnbp:
                    for g in range(cfg.NPOS):
                        if bake_g is not None and g != bake_g:
                            continue
                        with variant(g):
                            if use_cc:
                                emit_keys_qkv_cc(qkvps, g, str(g), l3, lnps, lnbp)
                            else:
                                emit_keys_qkv(qkvps, g, str(g), l3, lnps, lnbp)
                if use_cc:
                    nc.gpsimd.collective_compute(
                        "AllGather", mybir.AluOpType.bypass,
                        replica_groups=[[b * cfg.NPOS + i for i in range(cfg.NPOS)]
                                        for b in range(cfg.B)],
                        ins=[d_kvloc[:].opt()], outs=[d_kvgath[:].opt()])
                with tc.tile_pool(name=f"att{ri}", bufs=3) as att, \
                     tc.tile_pool(name=f"sps{ri}", bufs=3, space="PSUM") as spsum, \
                     tc.tile_pool(name=f"yps{ri}", bufs=2, space="PSUM") as ypsum:
                    for g in range(cfg.NPOS):
                        if bake_g is not None and g != bake_g:
                            continue
                        with variant(g):
                            if use_cc:
                                emit_readback(g, str(g))
                            emit_attention(g, str(g), att, spsum, ypsum)

            # ------- proj / LN2 / MLP (uniform across cores) -------
            with tc.tile_pool(name="l2c", bufs=1) as l2c, \
                 tc.tile_pool(name=f"mlpps{ri}", bufs=2, space="PSUM") as gpsum:
                ysel = l2c.tile([128, HC, R], F8)
                x1T = l2c.tile([128, EC, R], F32)
                x1b = l2c.tile([128, EC, R], BF16)
                x1f8 = l2c.tile([128, EC, R], F8)
                hT = l2c.tile([128, FC, R], BF16)

                nc.vector.tensor_tensor(
                    ysel[:], yT[:],
                    selb_s[:, None, :].to_broadcast([128, HC, R]), ALU.mult)

                for m in range(EC):
                    ps = gpsum.tile([128, 512], F32, tag="gp", name=f"psp{m}")
                    for c in range(EP):
                        nc.tensor.matmul(ps[:, :R], wp_s[:, 2 * c:2 * c + 2, ts(m, 128)],
                                         ysel[:, 2 * c:2 * c + 2, :],
                                         start=(c == 0), stop=False, perf_mode=DR)
                    # masked rows: += pv (x) (1-sel)   [rank-1; pv pre-scaled xS8]
                    nc.tensor.matmul(ps[:, :R], pv_s[0:1, ts(m, 128)], sel1m_s[:],
                                     start=False, stop=True)
                    nc.vector.scalar_tensor_tensor(x1T[:, m, :], ps[:, :R], 1.0 / S8,
                                                   xo_s[:, m, :], ALU.mult, ALU.add)
                    if flags["bproj"]:
                        nc.vector.tensor_scalar(x1T[:, m, :], x1T[:, m, :],
                                                bp_s[:, m:m + 1], None, ALU.add)
                    nc.gpsimd.tensor_copy(x1b[:, m, :], x1T[:, m, :])

                with tc.tile_pool(name="l3c", bufs=2) as l3c, \
                     tc.tile_pool(name=f"lnps2{ri}", bufs=3, space="PSUM") as lnps2, \
                     tc.tile_pool(name=f"lnbp2{ri}", bufs=1, space="PSUM") as lnbp2:
                    for cg0 in range(0, R, 512):
                        w = min(512, R - cg0)
                        ln_chunk(l3c, lnps2, lnbp2, x1b, cg0, w, (2, 3), f"2{cg0}",
                                 dst_of=lambda c, n0=cg0, w=w: x1f8[:, c, n0:n0 + w])

                for mg in range(FC // 4):
                    wfc_m4 = wstream.tile([128, 4 * EC, 128], F8, tag="wfc",
                                          name=f"wfc{mg}")
                    nc.sync.dma_start(
                        wfc_m4[:], d_wfc[mg].rearrange("p (f n) -> p f n", f=4 * EC))
                    for mi in range(4):
                        m = mg * 4 + mi
                        ps = gpsum.tile([128, 512], F32, tag="gp", name=f"psh{m}")
                        for c in range(EP):
                            nc.tensor.matmul(
                                ps[:, :R],
                                wfc_m4[:, mi * EC + 2 * c:mi * EC + 2 * c + 2, :],
                                x1f8[:, 2 * c:2 * c + 2, :],
                                start=(c == 0), stop=(c == EP - 1), perf_mode=DR)
                        if flags["bfc"]:
                            nc.scalar.activation(hT[:, m, :], ps[:, :R], AF.Silu,
                                                 bias=bfc_s[:, m:m + 1], scale=1.0 / S8)
                        else:
                            nc.scalar.activation(hT[:, m, :], ps[:, :R], AF.Silu,
                                                 scale=1.0 / S8)
                for m in range(EC):
                    wout_m = wstream2.tile([128, FC, 128], BF16, tag="wout", name=f"wout{m}")
                    nc.scalar.dma_start(wout_m[:], d_wout[m].rearrange("p (k n) -> p k n", k=FC))
                    ps = gpsum.tile([128, 512], F32, tag="gp", name=f"pso{m}")
                    for k in range(FC):
                        nc.tensor.matmul(ps[:, :R], wout_m[:, k, :], hT[:, k, :],
                                         start=(k == 0), stop=(k == FC - 1))
                    ot = wstream2.tile([128, 512], BF16, tag="ot", name=f"ot{m}")
                    nc.vector.tensor_tensor(ot[:, :R], ps[:, :R], x1T[:, m, :], ALU.add)
                    if flags["bout"]:
                        nc.vector.tensor_scalar(ot[:, :R], ot[:, :R],
                                                bout_s[:, m:m + 1], None, ALU.add)
                    nc.gpsimd.dma_start(out_r[:, m, :], ot[:, :R])

        if loop_n > 1:
            with tc.For_i(0, loop_n, 1):
                emit_body(0)
        else:
            for _ri in range(repeat):
                emit_body(_ri)

    nc.finalize()
    return nc


# ----------------------------------------------------------------------------
# host side: input prep / output assembly
# ----------------------------------------------------------------------------

def prepare_in_maps(cfg: Cfg, pairs, jmax, flags, inputs):
    """Build per-core input maps. Returns (in_maps, percore_blocks)."""
    x = np.asarray(inputs["x"], np.float32)
    w_qkv = np.asarray(inputs["w_qkv"], np.float32)
    b_qkv = np.asarray(inputs["b_qkv"], np.float32)
    w_proj = np.asarray(inputs["w_proj"], np.float32)
    b_proj = np.asarray(inputs["b_proj"], np.float32)
    w_fc = np.asarray(inputs["w_fc"], np.float32)
    b_fc = np.asarray(inputs["b_fc"], np.float32)
    w_out = np.asarray(inputs["w_out"], np.float32)
    b_out = np.asarray(inputs["b_out"], np.float32)
    ln1_s = np.asarray(inputs["ln1_scale"], np.float32)
    ln1_b = np.asarray(inputs["ln1_bias"], np.float32)
    ln2_s = np.asarray(inputs["ln2_scale"], np.float32)
    ln2_b = np.asarray(inputs["ln2_bias"], np.float32)
    mask_len = np.asarray(inputs["mask_len"]).astype(np.int64)

    E, L, H, D, BS = cfg.E, cfg.L, cfg.H, cfg.D, cfg.BS
    EC, FC = cfg.EC, cfg.FC
    KEYS = jmax * BS
    qscale = 1.0 / math.sqrt(D)

    # split qkv columns: col = h*3D + {0..D-1:q, D..2D-1:k, 2D..3D-1:v}
    # wq does NOT fold qscale (fp8 subnormals) -- applied on device readout
    wsplit = w_qkv.reshape(E, H, 3 * D)
    wq = np.ascontiguousarray(wsplit[:, :, 0:D].reshape(E, E))
    wk = np.ascontiguousarray(wsplit[:, :, D:2 * D].reshape(E, E))
    wv = np.ascontiguousarray(wsplit[:, :, 2 * D:3 * D].reshape(E, E))
    bsplit = b_qkv.reshape(H, 3 * D)
    bq = np.ascontiguousarray(bsplit[:, 0:D].reshape(E)) * qscale
    bk = np.ascontiguousarray(bsplit[:, D:2 * D].reshape(E))
    bv = np.ascontiguousarray(bsplit[:, 2 * D:3 * D].reshape(E))

    S8f = np.float32(S8)

    def chunked_w(w):  # [E, N] -> partition-major [128, EC*N] fp8 (pre-scaled)
        n = w.shape[1]
        return np.ascontiguousarray(
            (w * S8f).reshape(EC, 128, n).transpose(1, 0, 2)
            .reshape(128, EC * n)).astype(F8NP)

    def col_f32(v):    # [E or FF] -> [128, C]
        return np.ascontiguousarray(v.reshape(-1, 128).T).astype(np.float32)

    wq_c, wk_c, wv_c, wp_c = (chunked_w(w) for w in (wq, wk, wv, w_proj))
    wfc_c = np.ascontiguousarray(
        (w_fc * S8f).reshape(EC, 128, FC, 128).transpose(2, 1, 0, 3)
        .reshape(FC // 4, 4, 128, EC * 128).transpose(0, 2, 1, 3)
        .reshape(FC // 4, 128, 4 * EC * 128)
    ).astype(F8NP)
    wout_c = np.ascontiguousarray(
        w_out.reshape(FC, 128, EC, 128).transpose(2, 1, 0, 3)
        .reshape(EC, 128, FC * 128)
    ).astype(BF16NP)
    lnp = np.ascontiguousarray(np.stack(
        [col_f32(ln1_s), col_f32(ln1_b), col_f32(ln2_s), col_f32(ln2_b)]
    ).transpose(1, 0, 2))

    ki = np.arange(128)[:, None]
    qi = np.arange(BS)[None, :]
    masks = np.stack([(qi >= ki), (qi >= ki + 128)]).astype(BF16NP)

    # host-side vbar: attention output for fully-masked rows is the uniform
    # average of V over all L keys; pv = vbar @ w_proj folds into proj.
    mu = x.mean(-1, keepdims=True)
    var = ((x - mu) ** 2).mean(-1, keepdims=True)
    z = (x - mu) / np.sqrt(var + 1e-6) * ln1_s + ln1_b          # (B,L,E)
    vbar = z.mean(axis=1) @ wv + bv                              # (B,E)
    pv = vbar @ w_proj                                           # (B,E)

    shared = dict(
        wq=wq_c, wk=wk_c, wv=wv_c, wproj=wp_c, wfc=wfc_c, wout=wout_c,
        bq=col_f32(bq), bk=col_f32(bk),
        bv=(bv * S8f).reshape(1, E).astype(BF16NP),   # V psum is S8-scaled
        bproj=col_f32(b_proj), bfc=col_f32(b_fc), bout=col_f32(b_out),
        lnp=lnp, diagmasks=masks,
    )

    in_maps = []
    percore_blocks = []
    for c in range(cfg.n_cores):
        b = c // cfg.NPOS
        g = c % cfg.NPOS
        blocks = pairs[g]
        percore_blocks.append((b, blocks))
        xT = x[b].T  # [E, L]
        own_cols = np.concatenate(
            [np.arange(j * BS, (j + 1) * BS) for j in blocks])
        sel = (own_cols < mask_len[b]).astype(BF16NP)
        selb = np.broadcast_to(sel[None, :], (128, cfg.R))
        m = dict(shared)
        xk = xT[:, :KEYS]
        m["xT_full"] = np.ascontiguousarray(
            xk.reshape(EC, 128, KEYS).transpose(1, 0, 2).reshape(128, EC * KEYS)
        ).astype(BF16NP)
        xo = xT[:, own_cols]
        m["xT_own"] = np.ascontiguousarray(
            xo.reshape(EC, 128, -1).transpose(1, 0, 2).reshape(128, -1)).astype(BF16NP)
        m["selb"] = np.ascontiguousarray(selb)
        m["sel1m"] = np.ascontiguousarray(
            (1.0 - sel.astype(np.float32)).reshape(1, cfg.R)).astype(BF16NP)
        m["pv"] = np.ascontiguousarray(pv[b].reshape(1, E) * S8).astype(BF16NP)
        in_maps.append(m)
    return in_maps, percore_blocks


def assemble_output(cfg: Cfg, results, percore_blocks):
    out = np.zeros((cfg.B, cfg.L, cfg.E), np.float32)
    for c, res in enumerate(results):
        b, blocks = percore_blocks[c]
        oT = np.asarray(res["outT"], np.float32).reshape(
            128, cfg.EC, cfg.R).transpose(1, 0, 2).reshape(cfg.E, cfg.R)
        for s, j in enumerate(blocks):
            out[b, j * cfg.BS:(j + 1) * cfg.BS, :] = oT[:, s * cfg.BS:(s + 1) * cfg.BS].T
    return out


def make_flags(inputs):
    def nz(name):
        return bool(np.any(np.asarray(inputs[name]) != 0))
    return dict(
        bq=nz("b_qkv"), bk=nz("b_qkv"), bv=nz("b_qkv"),
        bproj=nz("b_proj"), bfc=nz("b_fc"), bout=nz("b_out"),
        ln1aff=bool(np.any(np.asarray(inputs["ln1_scale"]) != 1)
                    or np.any(np.asarray(inputs["ln1_bias"]) != 0)),
        ln2aff=bool(np.any(np.asarray(inputs["ln2_scale"]) != 1)
                    or np.any(np.asarray(inputs["ln2_bias"]) != 0)),
    )


_cached = {}


def kernel(**inputs) -> np.ndarray:
    cfg = Cfg()
    mask_len = np.asarray(inputs["mask_len"]).astype(np.int64)
    pairs, jmax = plan_blocks(cfg, mask_len)
    flags = make_flags(inputs)
    key = (tuple(map(tuple, pairs)), jmax, tuple(sorted(flags.items())), "v4")
    if key not in _cached:
        _cached[key] = build_program(cfg, pairs, jmax, flags, use_cc=False)
    nc = _cached[key]
    in_maps, percore_blocks = prepare_in_maps(cfg, pairs, jmax, flags, inputs)
    r = run_bass_kernel_spmd(nc, in_maps, core_ids=list(range(cfg.n_cores)))
    return assemble_output(cfg, r.results, percore_blocks)


if __name__ == "__main__":
    pass

